# revision 39
# baseline (speedup 1.0000x reference)
"""BinaryLinear (binarized nn.Linear) on 8 Trainium2 NeuronCores.

Reference op:
    alpha = mean(|W|, axis=1)                # per-output-row scale
    BW    = sign(W) * alpha                  # sign(0) := +1
    Y     = einsum('bsi,oi->bso', X, BW) + bias

Distribution: data-parallel over the batch dim (8 batches -> 1 per core).
Each core receives its batch slice of X pre-transposed (xT = [in, tok]),
split by k-range into an fp8(e4m3) part and a bf16 part, the full weight
in two bf16 layouts (wT = [in, out] for the stationary operand, w =
[out, in] for the per-row alpha reduction), and bias in fp32. Each core
computes the full [tok, out] output for its batch element, stored
transposed as [out, tok] in bf16; the host transposes/upcasts/stacks.

Precision plan (gate is rel_err < 2e-2): sign values are exact in every
dtype used (+-0.5). bf16 x contributes ~0.11% RMS, bf16 output ~0.11%.
FP8C k-chunks of the contraction run as fp8e4m3 DoubleRow matmuls
(2 MACs/cell/cycle, ~1.8x the bf16 row rate): each converted chunk-pair
replaces 2x216ns bf16 matmuls with one ~245ns DR matmul. e4m3 x adds
sqrt(FP8C/16)*2.55% RMS error: FP8C=4 measures ~1.3e-2, FP8C=6 ~1.6e-2.

Schedule per core:
  - matmul: K accumulated per PSUM bank; one out-chunk "wave" at a time
    on 4 PSUM banks (k-outer, t-inner: 4 consecutive matmuls share a
    stationary load), alternating bank halves so a wave's epilogues
    drain while the next wave's matmuls run. Waves 0+1 run interleaved
    k-outermost across all 8 banks so every arriving x slab unblocks 8
    matmuls, hiding the x stream (the first ~24us run under the chip's
    power-ramp duty throttle; DMA and PE both cap at ~50% there).
  - last wave runs t-outer/k-inner with per-tile epilogue+store so the
    output drains while the final k-sweeps run (shrinks the tail).
  - sign half-trick: s = (w >= 0) - 0.5 in {+0.5, -0.5}, one DVE op per
    dtype; the missing x2 is folded into alpha2 = 2*mean|W|.
  - alpha: DVE abs-accumulate reduce over natural-layout bf16 rows
    (fp32 accumulator), loaded after the x stream.
  - epilogue: ScalarE Identity(psum*alpha2 + bias) into a [128, T] bf16
    tile, stores on the ACT HW-DGE ring (SP ring stays pure loads).
"""

import os

import numpy as np

B, T, K, O = 8, 2048, 2048, 2048  # batch, tokens, in_features, out_features
P = 128          # SBUF partitions
KC = K // P      # 16 k-chunks
OC = O // P      # 16 out-chunk "waves"
TN = 512         # moving free-dim per matmul (PSUM bank limit in fp32)
TT = T // TN     # 4 token tiles

FP8C = 6         # leading k-chunks computed in fp8 DoubleRow (even, may be 0)
BFC = KC - FP8C  # trailing k-chunks computed in bf16
# waves 0/1 run while x still streams through the chip's ~20us power-ramp
# window (DMA capped at ~50% duty): they take a wider fp8 split so their
# critical stream shrinks from 7.5 to 6.5 MiB and their PE work drops ~3us.
# Global rel err 1.64e-2 -> 1.70e-2 (gate 2e-2).
F01 = 10         # fp8 k-chunks for waves 0/1 (even, >= FP8C)
XOFF01 = F01 - FP8C   # xb-tile index of wave-0/1's first bf16 chunk

N_CORES = 8

# Stashed by kernel() for test harnesses: BassKernelResults of the last run.
last_results = None

_cached_nc = None


def _build_program():
    global _cached_nc
    if _cached_nc is not None:
        return _cached_nc

    import concourse.tile as tile
    from concourse import bacc, bass_isa, mybir

    F32 = mybir.dt.float32
    BF16 = mybir.dt.bfloat16
    FP8 = mybir.dt.float8e4
    DR = mybir.MatmulPerfMode.DoubleRow
    IDENT = mybir.ActivationFunctionType.Identity
    ALU = mybir.AluOpType
    AX = mybir.AxisListType

    nc = bacc.Bacc("TRN2", target_bir_lowering=False, debug=False,
                   num_devices=N_CORES)

    xT = nc.dram_tensor("xT", [BFC * P, T], BF16, kind="ExternalInput").ap()
    wT = nc.dram_tensor("wT", [K, O], BF16, kind="ExternalInput").ap()
    # natural-layout rows only feed the per-row mean|W|: fp8 noise (~1.8%
    # RMS/elem) averages to ~0.04% on alpha, so ship them at half width
    w = nc.dram_tensor("w", [O, K], FP8, kind="ExternalInput").ap()
    b = nc.dram_tensor("b", [O], F32, kind="ExternalInput").ap()
    yT = nc.dram_tensor("yT", [O, T], BF16, kind="ExternalOutput").ap()
    scratch = nc.dram_tensor("scratch", [1, 1], F32, kind="Internal").ap()
    if FP8C:
        xT8 = nc.dram_tensor("xT8", [F01 * P, T], FP8,
                             kind="ExternalInput").ap()
        xT8_r = xT8.rearrange("(c p) t -> p c t", p=P)

    xT_r = xT.rearrange("(c p) t -> p c t", p=P)
    wT_r = wT.rearrange("(c p) o -> p c o", p=P)

    with tile.TileContext(nc) as tc:
        with (
            tc.tile_pool(name="xpool", bufs=1) as xpool,
            tc.tile_pool(name="wpool", bufs=3) as wpool,
            tc.tile_pool(name="spool", bufs=3) as spool,
            tc.tile_pool(name="npool", bufs=3) as npool,
            tc.tile_pool(name="apool", bufs=6) as apool,
            tc.tile_pool(name="opool", bufs=2) as opool,
            tc.tile_pool(name="const", bufs=1) as const,
            tc.tile_pool(name="psum", bufs=8, space="PSUM") as psum,
        ):
            def sign_prep(o, f8=FP8C):
                """Load + binarize the stationary operand for wave o."""
                wraw = wpool.tile([P, KC, P], BF16, tag="wraw",
                                  name=f"wraw{o}")
                nc.sync.dma_start(out=wraw, in_=wT_r[:, :, o * P:(o + 1) * P])
                sw8 = None
                if f8:
                    sw8 = spool.tile([P, f8, P], FP8, tag="sw8",
                                     name=f"sw8_{o}")
                    nc.vector.tensor_scalar(sw8, wraw[:, :f8, :], 0.0, 0.5,
                                            op0=ALU.is_ge, op1=ALU.subtract)
                sw = spool.tile([P, KC - f8, P], BF16, tag="sw",
                                name=f"sw{o}")
                nc.vector.tensor_scalar(sw, wraw[:, f8:, :], 0.0, 0.5,
                                        op0=ALU.is_ge, op1=ALU.subtract)
                return sw8, sw

            def alpha_prep(o):
                """alpha2 = 2*mean|W_row| from the natural-layout rows."""
                wn = npool.tile([P, K], FP8, tag="wn", name=f"wn{o}")
                nc.sync.dma_start(out=wn, in_=w[o * P:(o + 1) * P, :])
                asum = apool.tile([P, 1], F32, tag="asum", name=f"as{o}")
                nc.vector.tensor_reduce(asum, wn, axis=AX.X, op=ALU.add,
                                        apply_absolute_value=True)
                alpha2 = apool.tile([P, 1], F32, tag="alpha2", name=f"al{o}")
                nc.vector.tensor_scalar_mul(alpha2, asum, 2.0 / K)
                return alpha2

            # waves 0/1: stationary weights ahead of the x stream
            sws0 = sign_prep(0, F01)
            sws1 = sign_prep(1, F01)

            # prime the ScalarE during the idle startup: the first real
            # activation otherwise pays a lazy ~1.3us table load (and the
            # first store a ~0.6us HW-DGE init) right on the critical
            # alpha0 -> epilogue-0 -> wave-2 chain
            dummy = const.tile([1, 1], F32)
            nc.scalar.activation(dummy, sws0[1][0:1, 0, 0:1],
                                 mybir.ActivationFunctionType.Identity)
            nc.scalar.dma_start(out=scratch, in_=dummy)

            # resident x, in wave-0/1 consumption order: their bf16 opener
            # chunk first, then the fp8 pair-slabs (each unlocks 8 DR
            # matmuls), then their remaining bf16 chunks, then the chunks
            # only later waves read; alpha rows and bias ride after x.
            def x_load(c):
                xt = xpool.tile([P, T], BF16, tag=f"x{c}", name=f"xt{c}")
                nc.sync.dma_start(out=xt, in_=xT_r[:, c, :])
                return xt
            x_tiles = {}
            x_tiles[XOFF01] = x_load(XOFF01)
            x8 = None
            if FP8C:
                x8 = xpool.tile([P, F01, T], FP8, tag="xfp8")
                for v in range(F01 // 2):
                    nc.sync.dma_start(out=x8[:, 2 * v:2 * v + 2, :],
                                      in_=xT8_r[:, 2 * v:2 * v + 2, :])
            # the (small, fp8) alpha rows and bias ride here: waves 0/1 now
            # finish before the bf16 tail of the stream would deliver them,
            # and a late alpha0 stalls epilogue-0 -> wave-2's PSUM banks
            bias_sb = const.tile([P, OC], F32)
            nc.sync.dma_start(out=bias_sb,
                              in_=b.rearrange("(c p) -> p c", p=P))
            prepped = {0: (sws0, alpha_prep(0)), 1: (sws1, alpha_prep(1))}
            for c in range(XOFF01 + 1, BFC):
                x_tiles[c] = x_load(c)
            for c in range(XOFF01):
                x_tiles[c] = x_load(c)

            def psum_tiles(o):
                return [psum.tile([P, TN], F32, tag="ps", name=f"ps{o}_{t}")
                        for t in range(TT)]

            def mm_dr(ps_t, sw8, v, t):
                nc.tensor.matmul(
                    ps_t, lhsT=sw8[:, 2 * v:2 * v + 2, :],
                    rhs=x8[:, 2 * v:2 * v + 2, t * TN:(t + 1) * TN],
                    start=False, stop=False, perf_mode=DR)

            def mm_bf(ps_t, sw, c, t, start, stop, xoff=0):
                nc.tensor.matmul(
                    ps_t, lhsT=sw[:, c, :],
                    rhs=x_tiles[xoff + c][:, t * TN:(t + 1) * TN],
                    start=start, stop=stop)

            def epilogue(o, a2, ps):
                """4 activations into one [P, T] bf16 tile, one store."""
                ot = opool.tile([P, T], BF16, tag="ot", name=f"ot{o}")
                for t in range(TT):
                    nc.scalar.activation(ot[:, t * TN:(t + 1) * TN],
                                         ps[t], IDENT,
                                         bias=bias_sb[:, o:o + 1], scale=a2)
                # output DMAs ride the ACT HW-DGE ring: the SP ring's
                # in-order issue stream must stay pure loads
                nc.scalar.dma_start(out=yT[o * P:(o + 1) * P, :], in_=ot)

            # waves 0+1: x still streaming in, k-slab outermost so every
            # arriving x slab unblocks 8 matmuls (all psum banks). The
            # bf16 chunk-0 matmul opens each accumulation group: a plain
            # matmul's start=True is the proven-safe PSUM initializer.
            ps01 = [psum_tiles(0), psum_tiles(1)]
            for j in range(2):
                for t in range(TT):
                    mm_bf(ps01[j][t], prepped[j][0][1], 0, t,
                          start=True, stop=False, xoff=XOFF01)
            for v in range(F01 // 2):
                for j in range(2):
                    for t in range(TT):
                        mm_dr(ps01[j][t], prepped[j][0][0], v, t)
            for c in range(1, KC - F01):
                for j in range(2):
                    for t in range(TT):
                        mm_bf(ps01[j][t], prepped[j][0][1], c, t,
                              start=False, stop=c == KC - F01 - 1,
                              xoff=XOFF01)
            # weight prefetch for the next two waves queues behind x on SP
            prepped[2] = (sign_prep(2), alpha_prep(2))
            prepped[3] = (sign_prep(3), alpha_prep(3))
            epilogue(0, prepped.pop(0)[1], ps01[0])
            epilogue(1, prepped.pop(1)[1], ps01[1])

            # steady state: one wave per out-chunk on an alternating half
            # of PSUM (tag ring bufs=8 -> 2 waves in flight); k-outer /
            # t-inner so 4 consecutive matmuls share a stationary load and
            # the previous wave's epilogues overlap this wave's matmuls
            for o in range(2, OC - 1):
                (sw8, sw), a2 = prepped.pop(o)
                ps = psum_tiles(o)
                for t in range(TT):
                    mm_bf(ps[t], sw, 0, t, start=True, stop=False)
                for v in range(FP8C // 2):
                    for t in range(TT):
                        mm_dr(ps[t], sw8, v, t)
                for c in range(1, BFC):
                    for t in range(TT):
                        mm_bf(ps[t], sw, c, t,
                              start=False, stop=c == BFC - 1)
                if o + 2 < OC:
                    prepped[o + 2] = (sign_prep(o + 2), alpha_prep(o + 2))
                epilogue(o, a2, ps)

            # last wave: t-outer / k-inner with per-tile epilogue + store,
            # so 3 of 4 output tiles drain while later k-sweeps still run
            o = OC - 1
            (sw8, sw), a2 = prepped.pop(o)
            ps = psum_tiles(o)
            ot = opool.tile([P, T], BF16, tag="ot", name=f"ot{o}")
            for t in range(TT):
                mm_bf(ps[t], sw, 0, t, start=True, stop=False)
                for v in range(FP8C // 2):
                    mm_dr(ps[t], sw8, v, t)
                for c in range(1, BFC):
                    mm_bf(ps[t], sw, c, t,
                          start=False, stop=c == BFC - 1)
                nc.scalar.activation(ot[:, t * TN:(t + 1) * TN], ps[t], IDENT,
                                     bias=bias_sb[:, o:o + 1], scale=a2)
                nc.scalar.dma_start(
                    out=yT[o * P:(o + 1) * P, t * TN:(t + 1) * TN],
                    in_=ot[:, t * TN:(t + 1) * TN])

    nc.compile()
    _cached_nc = nc
    return nc


def _make_in_maps(x, weight, bias):
    import ml_dtypes
    bf16 = ml_dtypes.bfloat16
    from concourse import mybir
    fp8 = mybir.dt.np(mybir.dt.float8e4)
    wT = np.ascontiguousarray(weight.T).astype(bf16)
    w = np.ascontiguousarray(weight).astype(fp8)
    b = np.ascontiguousarray(bias)
    in_maps = []
    for core in range(N_CORES):
        xb = np.ascontiguousarray(x[core].T)  # [in, tok] fp32
        m = {"xT": xb[FP8C * P:].astype(bf16), "wT": wT, "w": w, "b": b}
        if FP8C:
            m["xT8"] = xb[:F01 * P].astype(fp8)
        in_maps.append(m)
    return in_maps


def _setup_trace_hooks():
    """Provide the antenv.axon_hooks NTFF hook missing from this image and
    skip the artifact bucket upload so trace=True works locally."""
    import sys
    import types

    try:
        from antenv.axon_hooks import get_axon_ntff_profile_hook  # noqa: F401
    except ImportError:
        mod = types.ModuleType("antenv.axon_hooks")
        _h = [None]
        mod.set_axon_ntff_profile_hook = lambda h: _h.__setitem__(0, h)
        mod.get_axon_ntff_profile_hook = lambda: _h[0]
        sys.modules["antenv.axon_hooks"] = mod
        import antenv

        antenv.axon_hooks = mod
        from trn_agent_boot.trn_boot import _ntff_profile_via_ctypes

        mod.set_axon_ntff_profile_hook(
            _ntff_profile_via_ctypes("/opt/axon/libaxon_pjrt.so"))

    import concourse.bass_utils as bu

    bu.upload_artifacts = lambda tmpdir: f"local://{tmpdir}"


def kernel(x: np.ndarray, weight: np.ndarray, bias: np.ndarray) -> np.ndarray:
    global last_results
    from concourse.bass_utils import run_bass_kernel_spmd

    x = np.asarray(x, dtype=np.float32)
    weight = np.asarray(weight, dtype=np.float32)
    bias = np.asarray(bias, dtype=np.float32)

    nc = _build_program()
    in_maps = _make_in_maps(x, weight, bias)
    trace = bool(int(os.environ.get("KERNEL_TRACE", "0")))
    trace_cores = None
    if trace:
        _setup_trace_hooks()
        tc_env = os.environ.get("KERNEL_TRACE_CORES", "")
        if tc_env:
            trace_cores = [int(c) for c in tc_env.split(",")]
    res = run_bass_kernel_spmd(nc, in_maps, list(range(N_CORES)), trace=trace,
                               trace_cores=trace_cores)
    last_results = res

    out = np.empty((B, T, O), dtype=np.float32)
    for core in range(N_CORES):
        out[core] = res.results[core]["yT"].T.astype(np.float32)
    return out


# revision 41
# speedup vs baseline: 1.0376x; 1.0376x over previous
"""BinaryLinear (binarized nn.Linear) on 8 Trainium2 NeuronCores.

Reference op:
    alpha = mean(|W|, axis=1)                # per-output-row scale
    BW    = sign(W) * alpha                  # sign(0) := +1
    Y     = einsum('bsi,oi->bso', X, BW) + bias

Distribution: data-parallel over the batch dim (8 batches -> 1 per core).
Each core receives its batch slice of X pre-transposed (xT = [in, tok]),
split by k-range into an fp8(e4m3) part and a bf16 part, the full weight
in two bf16 layouts (wT = [in, out] for the stationary operand, w =
[out, in] for the per-row alpha reduction), and bias in fp32. Each core
computes the full [tok, out] output for its batch element, stored
transposed as [out, tok] in bf16; the host transposes/upcasts/stacks.

Precision plan (gate is rel_err < 2e-2): sign values are exact in every
dtype used (+-0.5). bf16 x contributes ~0.11% RMS, bf16 output ~0.11%.
FP8C k-chunks of the contraction run as fp8e4m3 DoubleRow matmuls
(2 MACs/cell/cycle, ~1.8x the bf16 row rate): each converted chunk-pair
replaces 2x216ns bf16 matmuls with one ~245ns DR matmul. e4m3 x adds
sqrt(FP8C/16)*2.55% RMS error: FP8C=4 measures ~1.3e-2, FP8C=6 ~1.6e-2.

Schedule per core:
  - matmul: K accumulated per PSUM bank; one out-chunk "wave" at a time
    on 4 PSUM banks (k-outer, t-inner: 4 consecutive matmuls share a
    stationary load), alternating bank halves so a wave's epilogues
    drain while the next wave's matmuls run. Waves 0+1 run interleaved
    k-outermost across all 8 banks so every arriving x slab unblocks 8
    matmuls, hiding the x stream (the first ~24us run under the chip's
    power-ramp duty throttle; DMA and PE both cap at ~50% there).
  - last wave runs t-outer/k-inner with per-tile epilogue+store so the
    output drains while the final k-sweeps run (shrinks the tail).
  - sign half-trick: s = (w >= 0) - 0.5 in {+0.5, -0.5}, one DVE op per
    dtype; the missing x2 is folded into alpha2 = 2*mean|W|.
  - alpha: DVE abs-accumulate reduce over natural-layout bf16 rows
    (fp32 accumulator), loaded after the x stream.
  - epilogue: ScalarE Identity(psum*alpha2 + bias) into a [128, T] bf16
    tile, stores on the ACT HW-DGE ring (SP ring stays pure loads).
"""

import os

import numpy as np

B, T, K, O = 8, 2048, 2048, 2048  # batch, tokens, in_features, out_features
P = 128          # SBUF partitions
KC = K // P      # 16 k-chunks
OC = O // P      # 16 out-chunk "waves"
TN = 512         # moving free-dim per matmul (PSUM bank limit in fp32)
TT = T // TN     # 4 token tiles

FP8C = 6         # leading k-chunks computed in fp8 DoubleRow (even, may be 0)
BFC = KC - FP8C  # trailing k-chunks computed in bf16
# Waves 0/1 may take a wider fp8 split (F01 > FP8C) to shrink their
# critical stream through the chip's ~20us power-ramp window; measured a
# net loss at F01=10 (the duplicated bf16 chunks re-stall waves 2/3 and
# re-trigger the duty throttle), so it stays symmetric.
F01 = 6          # fp8 k-chunks for waves 0/1 (even, >= FP8C)
XOFF01 = F01 - FP8C   # xb-tile index of wave-0/1's first bf16 chunk

N_CORES = 8

# Stashed by kernel() for test harnesses: BassKernelResults of the last run.
last_results = None

_cached_nc = None


def _build_program():
    global _cached_nc
    if _cached_nc is not None:
        return _cached_nc

    import concourse.tile as tile
    from concourse import bacc, bass_isa, mybir

    F32 = mybir.dt.float32
    BF16 = mybir.dt.bfloat16
    FP8 = mybir.dt.float8e4
    DR = mybir.MatmulPerfMode.DoubleRow
    IDENT = mybir.ActivationFunctionType.Identity
    ALU = mybir.AluOpType
    AX = mybir.AxisListType

    nc = bacc.Bacc("TRN2", target_bir_lowering=False, debug=False,
                   num_devices=N_CORES)

    xT = nc.dram_tensor("xT", [BFC * P, T], BF16, kind="ExternalInput").ap()
    wT = nc.dram_tensor("wT", [K, O], BF16, kind="ExternalInput").ap()
    # natural-layout rows only feed the per-row mean|W|: fp8 noise (~1.8%
    # RMS/elem) averages to ~0.04% on alpha, so ship them at half width
    w = nc.dram_tensor("w", [O, K], FP8, kind="ExternalInput").ap()
    b = nc.dram_tensor("b", [O], F32, kind="ExternalInput").ap()
    yT = nc.dram_tensor("yT", [O, T], BF16, kind="ExternalOutput").ap()
    scratch = nc.dram_tensor("scratch", [1, 1], F32, kind="Internal").ap()
    if FP8C:
        xT8 = nc.dram_tensor("xT8", [F01 * P, T], FP8,
                             kind="ExternalInput").ap()
        xT8_r = xT8.rearrange("(c p) t -> p c t", p=P)

    xT_r = xT.rearrange("(c p) t -> p c t", p=P)
    wT_r = wT.rearrange("(c p) o -> p c o", p=P)

    with tile.TileContext(nc) as tc:
        with (
            tc.tile_pool(name="xpool", bufs=1) as xpool,
            tc.tile_pool(name="wpool", bufs=3) as wpool,
            tc.tile_pool(name="spool", bufs=3) as spool,
            tc.tile_pool(name="npool", bufs=3) as npool,
            tc.tile_pool(name="apool", bufs=6) as apool,
            tc.tile_pool(name="opool", bufs=2) as opool,
            tc.tile_pool(name="const", bufs=1) as const,
            tc.tile_pool(name="psum", bufs=8, space="PSUM") as psum,
        ):
            def sign_prep(o, f8=FP8C):
                """Load + binarize the stationary operand for wave o."""
                wraw = wpool.tile([P, KC, P], BF16, tag="wraw",
                                  name=f"wraw{o}")
                nc.sync.dma_start(out=wraw, in_=wT_r[:, :, o * P:(o + 1) * P])
                sw8 = None
                if f8:
                    sw8 = spool.tile([P, f8, P], FP8, tag="sw8",
                                     name=f"sw8_{o}")
                    nc.vector.tensor_scalar(sw8, wraw[:, :f8, :], 0.0, 0.5,
                                            op0=ALU.is_ge, op1=ALU.subtract)
                sw = spool.tile([P, KC - f8, P], BF16, tag="sw",
                                name=f"sw{o}")
                nc.vector.tensor_scalar(sw, wraw[:, f8:, :], 0.0, 0.5,
                                        op0=ALU.is_ge, op1=ALU.subtract)
                return sw8, sw

            def alpha_prep(o):
                """alpha2 = 2*mean|W_row| from the natural-layout rows."""
                wn = npool.tile([P, K], FP8, tag="wn", name=f"wn{o}")
                nc.sync.dma_start(out=wn, in_=w[o * P:(o + 1) * P, :])
                asum = apool.tile([P, 1], F32, tag="asum", name=f"as{o}")
                nc.vector.tensor_reduce(asum, wn, axis=AX.X, op=ALU.add,
                                        apply_absolute_value=True)
                alpha2 = apool.tile([P, 1], F32, tag="alpha2", name=f"al{o}")
                nc.vector.tensor_scalar_mul(alpha2, asum, 2.0 / K)
                return alpha2

            # waves 0/1: stationary weights ahead of the x stream
            sws0 = sign_prep(0, F01)
            sws1 = sign_prep(1, F01)

            # prime the ScalarE during the idle startup: the first real
            # activation otherwise pays a lazy ~1.3us table load (and the
            # first store a ~0.6us HW-DGE init) right on the critical
            # alpha0 -> epilogue-0 -> wave-2 chain
            dummy = const.tile([1, 1], F32)
            nc.scalar.activation(dummy, sws0[1][0:1, 0, 0:1],
                                 mybir.ActivationFunctionType.Identity)
            nc.scalar.dma_start(out=scratch, in_=dummy)

            # resident x, in wave-0/1 consumption order: their bf16 opener
            # chunk first, then the fp8 pair-slabs (each unlocks 8 DR
            # matmuls), then their remaining bf16 chunks, then the chunks
            # only later waves read; alpha rows and bias ride after x.
            def x_load(c):
                xt = xpool.tile([P, T], BF16, tag=f"x{c}", name=f"xt{c}")
                nc.sync.dma_start(out=xt, in_=xT_r[:, c, :])
                return xt
            x_tiles = {}
            x_tiles[XOFF01] = x_load(XOFF01)
            x8 = None
            if FP8C:
                x8 = xpool.tile([P, F01, T], FP8, tag="xfp8")
                for v in range(F01 // 2):
                    nc.sync.dma_start(out=x8[:, 2 * v:2 * v + 2, :],
                                      in_=xT8_r[:, 2 * v:2 * v + 2, :])
            for c in range(XOFF01 + 1, BFC):
                x_tiles[c] = x_load(c)
            for c in range(XOFF01):
                x_tiles[c] = x_load(c)
            bias_sb = const.tile([P, OC], F32)
            nc.sync.dma_start(out=bias_sb,
                              in_=b.rearrange("(c p) -> p c", p=P))
            prepped = {0: (sws0, alpha_prep(0)), 1: (sws1, alpha_prep(1))}

            def psum_tiles(o):
                return [psum.tile([P, TN], F32, tag="ps", name=f"ps{o}_{t}")
                        for t in range(TT)]

            def mm_dr(ps_t, sw8, v, t):
                nc.tensor.matmul(
                    ps_t, lhsT=sw8[:, 2 * v:2 * v + 2, :],
                    rhs=x8[:, 2 * v:2 * v + 2, t * TN:(t + 1) * TN],
                    start=False, stop=False, perf_mode=DR)

            def mm_bf(ps_t, sw, c, t, start, stop, xoff=0):
                nc.tensor.matmul(
                    ps_t, lhsT=sw[:, c, :],
                    rhs=x_tiles[xoff + c][:, t * TN:(t + 1) * TN],
                    start=start, stop=stop)

            def epilogue(o, a2, ps):
                """4 activations into one [P, T] bf16 tile, one store."""
                ot = opool.tile([P, T], BF16, tag="ot", name=f"ot{o}")
                for t in range(TT):
                    nc.scalar.activation(ot[:, t * TN:(t + 1) * TN],
                                         ps[t], IDENT,
                                         bias=bias_sb[:, o:o + 1], scale=a2)
                # output DMAs ride the ACT HW-DGE ring: the SP ring's
                # in-order issue stream must stay pure loads
                nc.scalar.dma_start(out=yT[o * P:(o + 1) * P, :], in_=ot)

            # waves 0+1: x still streaming in, k-slab outermost so every
            # arriving x slab unblocks 8 matmuls (all psum banks). The
            # bf16 chunk-0 matmul opens each accumulation group: a plain
            # matmul's start=True is the proven-safe PSUM initializer.
            ps01 = [psum_tiles(0), psum_tiles(1)]
            for j in range(2):
                for t in range(TT):
                    mm_bf(ps01[j][t], prepped[j][0][1], 0, t,
                          start=True, stop=False, xoff=XOFF01)
            for v in range(F01 // 2):
                for j in range(2):
                    for t in range(TT):
                        mm_dr(ps01[j][t], prepped[j][0][0], v, t)
            for c in range(1, KC - F01):
                for j in range(2):
                    for t in range(TT):
                        mm_bf(ps01[j][t], prepped[j][0][1], c, t,
                              start=False, stop=c == KC - F01 - 1,
                              xoff=XOFF01)
            # weight prefetch for the next two waves queues behind x on SP
            prepped[2] = (sign_prep(2), alpha_prep(2))
            prepped[3] = (sign_prep(3), alpha_prep(3))
            epilogue(0, prepped.pop(0)[1], ps01[0])
            epilogue(1, prepped.pop(1)[1], ps01[1])

            # steady state: one wave per out-chunk on an alternating half
            # of PSUM (tag ring bufs=8 -> 2 waves in flight); k-outer /
            # t-inner so 4 consecutive matmuls share a stationary load and
            # the previous wave's epilogues overlap this wave's matmuls
            for o in range(2, OC - 1):
                (sw8, sw), a2 = prepped.pop(o)
                ps = psum_tiles(o)
                for t in range(TT):
                    mm_bf(ps[t], sw, 0, t, start=True, stop=False)
                for v in range(FP8C // 2):
                    for t in range(TT):
                        mm_dr(ps[t], sw8, v, t)
                for c in range(1, BFC):
                    for t in range(TT):
                        mm_bf(ps[t], sw, c, t,
                              start=False, stop=c == BFC - 1)
                if o + 2 < OC:
                    prepped[o + 2] = (sign_prep(o + 2), alpha_prep(o + 2))
                epilogue(o, a2, ps)

            # last wave: t-outer / k-inner with per-tile epilogue + store,
            # so 3 of 4 output tiles drain while later k-sweeps still run
            o = OC - 1
            (sw8, sw), a2 = prepped.pop(o)
            ps = psum_tiles(o)
            ot = opool.tile([P, T], BF16, tag="ot", name=f"ot{o}")
            for t in range(TT):
                mm_bf(ps[t], sw, 0, t, start=True, stop=False)
                for v in range(FP8C // 2):
                    mm_dr(ps[t], sw8, v, t)
                for c in range(1, BFC):
                    mm_bf(ps[t], sw, c, t,
                          start=False, stop=c == BFC - 1)
                nc.scalar.activation(ot[:, t * TN:(t + 1) * TN], ps[t], IDENT,
                                     bias=bias_sb[:, o:o + 1], scale=a2)
                nc.scalar.dma_start(
                    out=yT[o * P:(o + 1) * P, t * TN:(t + 1) * TN],
                    in_=ot[:, t * TN:(t + 1) * TN])

    nc.compile()
    _cached_nc = nc
    return nc


def _make_in_maps(x, weight, bias):
    import ml_dtypes
    bf16 = ml_dtypes.bfloat16
    from concourse import mybir
    fp8 = mybir.dt.np(mybir.dt.float8e4)
    wT = np.ascontiguousarray(weight.T).astype(bf16)
    w = np.ascontiguousarray(weight).astype(fp8)
    b = np.ascontiguousarray(bias)
    in_maps = []
    for core in range(N_CORES):
        xb = np.ascontiguousarray(x[core].T)  # [in, tok] fp32
        m = {"xT": xb[FP8C * P:].astype(bf16), "wT": wT, "w": w, "b": b}
        if FP8C:
            m["xT8"] = xb[:F01 * P].astype(fp8)
        in_maps.append(m)
    return in_maps


def _setup_trace_hooks():
    """Provide the antenv.axon_hooks NTFF hook missing from this image and
    skip the artifact bucket upload so trace=True works locally."""
    import sys
    import types

    try:
        from antenv.axon_hooks import get_axon_ntff_profile_hook  # noqa: F401
    except ImportError:
        mod = types.ModuleType("antenv.axon_hooks")
        _h = [None]
        mod.set_axon_ntff_profile_hook = lambda h: _h.__setitem__(0, h)
        mod.get_axon_ntff_profile_hook = lambda: _h[0]
        sys.modules["antenv.axon_hooks"] = mod
        import antenv

        antenv.axon_hooks = mod
        from trn_agent_boot.trn_boot import _ntff_profile_via_ctypes

        mod.set_axon_ntff_profile_hook(
            _ntff_profile_via_ctypes("/opt/axon/libaxon_pjrt.so"))

    import concourse.bass_utils as bu

    bu.upload_artifacts = lambda tmpdir: f"local://{tmpdir}"


def kernel(x: np.ndarray, weight: np.ndarray, bias: np.ndarray) -> np.ndarray:
    global last_results
    from concourse.bass_utils import run_bass_kernel_spmd

    x = np.asarray(x, dtype=np.float32)
    weight = np.asarray(weight, dtype=np.float32)
    bias = np.asarray(bias, dtype=np.float32)

    nc = _build_program()
    in_maps = _make_in_maps(x, weight, bias)
    trace = bool(int(os.environ.get("KERNEL_TRACE", "0")))
    trace_cores = None
    if trace:
        _setup_trace_hooks()
        tc_env = os.environ.get("KERNEL_TRACE_CORES", "")
        if tc_env:
            trace_cores = [int(c) for c in tc_env.split(",")]
    res = run_bass_kernel_spmd(nc, in_maps, list(range(N_CORES)), trace=trace,
                               trace_cores=trace_cores)
    last_results = res

    out = np.empty((B, T, O), dtype=np.float32)
    for core in range(N_CORES):
        out[core] = res.results[core]["yT"].T.astype(np.float32)
    return out
